# revision 3
# baseline (speedup 1.0000x reference)
"""CRF-RNN mean-field iteration kernel for Trainium2 (8 NeuronCores).

Math (per batch b, NITERS=5):
    D_norm = W / W.sum(axis=1, keepdims)          # row-normalized affinity [n, n]
    qVals  = uniqs = seg.reshape(d, n)
    loop:  Q = softmax(qVals, axis=0)             # over class dim d=21
           seg_diff   = Q @ D_norm^T              # [d, n]
           seg_update = weights @ seg_diff
           qVals      = uniqs - seg_update

Sharding: batch b -> core pair (2b, 2b+1); each core owns half the output
positions (m rows of W). The contraction runs over all n, so W^T (contraction
index on partitions) is built on-device via PE transpose-matmuls, cast to
fp16, and kept resident in SBUF across all 5 iterations -- W is read from HBM
exactly once. Row-normalization is folded into the tiny matmul outputs. Per
iteration the pair exchanges its half of softmax(Q) (86 KB fp16) via a
pairwise AllGather; the instruction stream is identical on all cores (SPMD):
all own/partner asymmetry lives in host-side input permutations and a tiny
select-mask input.
"""

import os
import sys

for _p in ("/opt/trn_rl_repo",):
    if _p not in sys.path:
        sys.path.insert(0, _p)

import numpy as np

BS, D, RC = 4, 21, 64
N = RC * RC      # 4096 positions
NH = N // 2      # 2048 positions per core (own half)
NT = 32          # 128-wide position tiles (global)
NTO = 16         # own tiles
SLABS = 16       # own-half m slabs of 128 rows
NITERS = int(os.environ.get("CRF_NITERS", "5"))
NCORES = 8
RG = [[0, 1], [2, 3], [4, 5], [6, 7]]

LAST_EXEC_NS = None
_CACHE = {}


def _install_ntff_hook():
    """Best-effort registration of the axon NTFF profile hook (image antenv
    lacks axon_hooks, so trn_boot could not register it)."""
    try:
        import types

        if "antenv.axon_hooks" in sys.modules:
            return
        holder = [None]
        m = types.ModuleType("antenv.axon_hooks")
        m.set_axon_ntff_profile_hook = lambda h: holder.__setitem__(0, h)
        m.get_axon_ntff_profile_hook = lambda: holder[0]
        sys.modules["antenv.axon_hooks"] = m
        import antenv

        antenv.axon_hooks = m
        from trn_agent_boot.trn_boot import _ntff_profile_via_ctypes

        m.set_axon_ntff_profile_hook(
            _ntff_profile_via_ctypes("/opt/axon/libaxon_pjrt.so")
        )
    except Exception:
        pass


def _build(niters):
    from concourse import bacc, bass, tile, mybir, masks

    fp32, fp16 = mybir.dt.float32, mybir.dt.float16
    AF = mybir.ActivationFunctionType
    ALU = mybir.AluOpType

    nc = bacc.Bacc(None, target_bir_lowering=False)

    w_in = nc.dram_tensor("w", (NH, N), fp32, kind="ExternalInput")
    segt_in = nc.dram_tensor("segt", (128, NT, D), fp32, kind="ExternalInput")
    wt_in = nc.dram_tensor("wt", (D, D), fp32, kind="ExternalInput")
    sel_in = nc.dram_tensor("sel", (128, 2), fp32, kind="ExternalInput")
    out_t = nc.dram_tensor("out", (128, NTO, D), fp32, kind="ExternalOutput")

    n_ex = max(0, niters - 1)
    cc_ins = [
        nc.dram_tensor(f"cc_in{k}", (128, NTO * D), fp16, kind="Internal")
        for k in range(n_ex)
    ]
    cc_outs = [
        nc.dram_tensor(f"cc_out{k}", (2, 128, NTO * D), fp16, kind="Internal")
        for k in range(n_ex)
    ]

    with tile.TileContext(nc) as tc:
        with (
            tc.tile_pool(name="wt_res", bufs=1) as wt_res,
            tc.tile_pool(name="slab", bufs=3) as slabp,
            tc.tile_pool(name="state", bufs=1) as state,
            tc.tile_pool(name="qt", bufs=2) as qtp,
            tc.tile_pool(name="work", bufs=2) as work,
            tc.tile_pool(name="ps_mm", bufs=1, space=bass.MemorySpace.PSUM) as ps_mm,
            tc.tile_pool(name="ps_misc", bufs=2, space=bass.MemorySpace.PSUM) as ps_misc,
            tc.tile_pool(name="ps_rs", bufs=1, space=bass.MemorySpace.PSUM) as ps_rs,
        ):
            # ---- constants / small inputs ------------------------------
            ident16 = state.tile([128, 128], fp16)
            masks.make_identity(nc, ident16[:])
            ones1 = state.tile([128, 1], fp16)
            nc.gpsimd.memset(ones1[:], 1.0)
            ones21 = state.tile([1, D], fp32)
            nc.gpsimd.memset(ones21[:], 1.0)
            zbias = state.tile([128, 1], fp32)
            nc.gpsimd.memset(zbias[:], 0.0)

            wt32 = state.tile([D, D], fp32)
            nc.sync.dma_start(wt32[:], wt_in[:])
            wt16 = state.tile([D, D], fp16)
            nc.vector.tensor_copy(wt16[:], wt32[:])

            segt = state.tile([128, NT, D], fp32)
            nc.sync.dma_start(segt[:], segt_in[:])
            selt = state.tile([128, 2], fp32)
            nc.sync.dma_start(selt[:], sel_in[:])

            # ---- initial Q = softmax(uniqs) over all 32 tiles ----------
            ex0 = state.tile([128, NT, D], fp32)
            nc.scalar.activation(ex0[:], segt[:], AF.Exp, bias=zbias[:])
            ssum0 = state.tile([128, NT], fp32)
            nc.vector.reduce_sum(ssum0[:], ex0[:], axis=mybir.AxisListType.X)
            srecip0 = state.tile([128, NT], fp32)
            nc.vector.reciprocal(srecip0[:], ssum0[:])
            qt_own = qtp.tile([128, NTO, D], fp16, tag="qt_own")
            qt_par = qtp.tile([128, NTO, D], fp16, tag="qt_par")
            for t in range(NT):
                dst = qt_own if t < NTO else qt_par
                nc.vector.tensor_scalar_mul(
                    dst[:, t % NTO, :], ex0[:, t, :], srecip0[:, t : t + 1]
                )

            # ---- phase 1: load W half (cast fp16 in DMA) + transpose ---
            # wt_mc[mc][:, nt, j] = W^T[128*nt + p, 512*mc + j]  (own-first
            # permuted n on partitions, local m on free dim)
            wt_mc = [
                wt_res.tile([128, NT, 512], fp16, tag=f"wtr{mc}", name=f"wt_mc{mc}")
                for mc in range(4)
            ]
            for ms in range(SLABS):
                w16 = slabp.tile([128, N], fp16, tag="w16")
                nc.gpsimd.dma_start(w16[:], w_in[ms * 128 : (ms + 1) * 128, :])
                mc, col = ms // 4, (ms % 4) * 128
                for g in range(8):
                    ptp = ps_misc.tile([128, 512], fp32, tag="misc")
                    for k2 in range(4):
                        nt = 4 * g + k2
                        nc.tensor.matmul(
                            ptp[:, k2 * 128 : (k2 + 1) * 128],
                            w16[:, nt * 128 : (nt + 1) * 128],
                            ident16[:],
                            start=True,
                            stop=True,
                        )
                    dst = wt_mc[mc][:, 4 * g : 4 * g + 4, col : col + 128]
                    src = ptp[:].rearrange("p (a b) -> p a b", a=4)
                    if g % 2 == 0:
                        nc.vector.tensor_copy(dst, src)
                    else:
                        nc.scalar.activation(dst, src, AF.Copy)

            # ---- phase 2: row sums via ones-matmul + reciprocal bcast --
            rs_sb = state.tile([1, NH], fp32)
            for mc in range(4):
                prs = ps_rs.tile([1, 512], fp32, tag="rs")
                for nt in range(NT):
                    nc.tensor.matmul(
                        prs[:],
                        ones1[:],
                        wt_mc[mc][:, nt, :],
                        start=(nt == 0),
                        stop=(nt == NT - 1),
                    )
                nc.vector.tensor_copy(rs_sb[:, mc * 512 : (mc + 1) * 512], prs[:])
            rs_rec = state.tile([1, NH], fp32)
            nc.vector.reciprocal(rs_rec[:], rs_sb[:])
            # broadcast across the 21 class partitions via K=1 matmul
            rsb = state.tile([D, NH], fp32)
            for mc in range(4):
                pb = ps_misc.tile([128, 512], fp32, tag="misc")
                nc.tensor.matmul(
                    pb[:D, :],
                    ones21[:],
                    rs_rec[:, mc * 512 : (mc + 1) * 512],
                    start=True,
                    stop=True,
                )
                nc.vector.tensor_copy(rsb[:, mc * 512 : (mc + 1) * 512], pb[:D, :])

            # ---- phase 3: the 5 mean-field iterations ------------------
            for it in range(niters):
                # seg_diff^T accumulation: psum[d, m] += QT[nt].T @ Wt[nt]
                pP = ps_mm.tile([D, NH], fp32, tag="pp")
                for idx in range(NT):
                    src = qt_own if idx < NTO else qt_par
                    lhs = src[:, idx % NTO, :]
                    for mc in range(4):
                        nc.tensor.matmul(
                            pP[:, mc * 512 : (mc + 1) * 512],
                            lhs,
                            wt_mc[mc][:, idx, :],
                            start=(idx == 0),
                            stop=(idx == NT - 1),
                        )
                # evacuate with row-norm scale -> fp16
                ps16 = work.tile([D, NH], fp16, tag="ps16")
                for mc in range(4):
                    sl = slice(mc * 512, (mc + 1) * 512)
                    nc.vector.tensor_tensor(
                        ps16[:, sl], pP[:, sl], rsb[:, sl], ALU.mult
                    )
                # seg_update^T for own tiles: psum[m128, d] = Ps.T @ weights^T
                pUT = ps_misc.tile([128, NTO * D], fp32, tag="misc")
                for j in range(NTO):
                    nc.tensor.matmul(
                        pUT[:, j * D : (j + 1) * D],
                        ps16[:, j * 128 : (j + 1) * 128],
                        wt16[:],
                        start=True,
                        stop=True,
                    )
                # qVals = uniqs - seg_update (own half)
                qv = work.tile([128, NTO, D], fp32, tag="qv")
                nc.vector.tensor_tensor(
                    qv[:],
                    segt[:, 0:NTO, :],
                    pUT[:].rearrange("p (a b) -> p a b", a=NTO),
                    ALU.subtract,
                )
                if it == niters - 1:
                    nc.sync.dma_start(out_t[:], qv[:])
                    continue
                # softmax over d (free dim) without max-subtraction
                exq = work.tile([128, NTO, D], fp32, tag="exq")
                nc.scalar.activation(exq[:], qv[:], AF.Exp, bias=zbias[:])
                ssum = work.tile([128, NTO], fp32, tag="ssum")
                nc.vector.reduce_sum(ssum[:], exq[:], axis=mybir.AxisListType.X)
                srec = work.tile([128, NTO], fp32, tag="srec")
                nc.vector.reciprocal(srec[:], ssum[:])
                qt_own = qtp.tile([128, NTO, D], fp16, tag="qt_own")
                for j in range(NTO):
                    nc.vector.tensor_scalar_mul(
                        qt_own[:, j, :], exq[:, j, :], srec[:, j : j + 1]
                    )
                # pairwise exchange of the own Q half
                nc.sync.dma_start(cc_ins[it][:], qt_own[:])
                nc.gpsimd.collective_compute(
                    "AllGather",
                    ALU.bypass,
                    replica_groups=RG,
                    ins=[cc_ins[it][:].opt()],
                    outs=[cc_outs[it][:].opt()],
                )
                g0 = work.tile([128, NTO * D], fp16, tag="g0")
                g1 = work.tile([128, NTO * D], fp16, tag="g1")
                nc.sync.dma_start(g0[:], cc_outs[it][0][:])
                nc.sync.dma_start(g1[:], cc_outs[it][1][:])
                t0 = work.tile([128, NTO * D], fp16, tag="t0")
                t1 = work.tile([128, NTO * D], fp16, tag="t1")
                nc.vector.tensor_scalar_mul(t0[:], g0[:], selt[:, 0:1])
                nc.vector.tensor_scalar_mul(t1[:], g1[:], selt[:, 1:2])
                qt_par = qtp.tile([128, NTO, D], fp16, tag="qt_par")
                nc.vector.tensor_tensor(
                    qt_par[:],
                    t0[:].rearrange("p (a b) -> p a b", a=NTO),
                    t1[:].rearrange("p (a b) -> p a b", a=NTO),
                    ALU.add,
                )

    nc.compile()
    return nc


def _get_nc(niters):
    if niters not in _CACHE:
        _CACHE[niters] = _build(niters)
    return _CACHE[niters]


def kernel(seg, W, weights):
    global LAST_EXEC_NS
    assert seg.shape == (BS, D, RC, RC) and W.shape == (BS, N, N)
    trace = bool(os.environ.get("BASS_TRACE"))
    if trace:
        _install_ntff_hook()

    from concourse.bass_utils import run_bass_kernel_spmd

    nc = _get_nc(NITERS)

    seg32 = np.ascontiguousarray(seg, dtype=np.float32)
    W32 = np.ascontiguousarray(W, dtype=np.float32)
    wt_np = np.ascontiguousarray(weights.T, dtype=np.float32)

    in_maps = []
    for c in range(NCORES):
        b, h = c // 2, c % 2
        own = slice(NH * h, NH * h + NH)
        par = slice(NH * (1 - h), NH * (1 - h) + NH)
        Wb = W32[b]
        w_np = np.ascontiguousarray(
            np.concatenate([Wb[own, own], Wb[own, par]], axis=1)
        )
        st = seg32[b].reshape(D, N).T  # [n, d]
        st_perm = np.concatenate([st[own], st[par]], axis=0)
        segt_np = np.ascontiguousarray(
            st_perm.reshape(NT, 128, D).transpose(1, 0, 2)
        )
        sel_np = np.zeros((128, 2), np.float32)
        sel_np[:, 0] = float(h)       # pick gather slot (1-h) = partner
        sel_np[:, 1] = float(1 - h)
        in_maps.append({"w": w_np, "segt": segt_np, "wt": wt_np, "sel": sel_np})

    res = run_bass_kernel_spmd(
        nc, in_maps, core_ids=list(range(NCORES)), trace=trace
    )
    LAST_EXEC_NS = res.exec_time_ns

    out = np.empty((BS, D, N), np.float32)
    for c in range(NCORES):
        b, h = c // 2, c % 2
        qv = res.results[c]["out"]  # [128, NTO, D]
        block = qv.transpose(2, 1, 0).reshape(D, NH)
        out[b][:, NH * h : NH * h + NH] = block
    return out.reshape(BS, D, RC, RC)


if __name__ == "__main__":
    rng = np.random.default_rng(0)
    seg = rng.standard_normal((BS, D, RC, RC)).astype(np.float32)
    W = rng.random((BS, N, N), dtype=np.float32)
    weights = rng.standard_normal((D, D)).astype(np.float32)
    out = kernel(seg=seg, W=W, weights=weights)
    print("out", out.shape, out.dtype, float(np.abs(out).mean()))


# revision 4
# speedup vs baseline: 1.2183x; 1.2183x over previous
"""CRF-RNN mean-field iteration kernel for Trainium2 (8 NeuronCores).

Math (per batch b, NITERS=5):
    D_norm = W / W.sum(axis=1, keepdims)          # row-normalized affinity [n, n]
    qVals  = uniqs = seg.reshape(d, n)
    loop:  Q = softmax(qVals, axis=0)             # over class dim d=21
           seg_diff   = Q @ D_norm^T              # [d, n]
           seg_update = weights @ seg_diff
           qVals      = uniqs - seg_update

Sharding: batch b -> core pair (2b, 2b+1); each core owns half the output
positions (m rows of W). The contraction runs over all n, so W^T (contraction
index on partitions) is built on-device via PE transpose-matmuls against an
identity, quantized to fp8-e4m3, and kept resident in SBUF across all 5
iterations -- W is read from HBM exactly once. The main matmuls run in fp8
DoubleRow mode (256-wide contraction per pass). Row-normalization (1/rowsum,
accumulated for free during the fp32->fp8 cast on the Scalar engine) is
applied per-partition to the tiny seg_update output. Per iteration the pair
exchanges its half of softmax(Q) (64 KB fp8) via a pairwise AllGather; the
instruction stream is identical on all cores (SPMD): all own/partner
asymmetry lives in host-side input permutations and a tiny select-mask input.
"""

import os
import sys

for _p in ("/opt/trn_rl_repo",):
    if _p not in sys.path:
        sys.path.insert(0, _p)

import numpy as np

BS, D, RC = 4, 21, 64
N = RC * RC       # 4096 positions
NH = N // 2       # 2048 positions per core (own half)
NT = 32           # 128-wide position tiles (global)
NTO = 16          # own tiles
NT2 = 16          # 256-wide fp8 pair tiles (global)
SLABS = 16        # own-half m slabs of 128 rows
QPAD = 32         # class-dim padding for fp8 DoubleRow lhsT stride
NITERS = int(os.environ.get("CRF_NITERS", "5"))
FP8 = os.environ.get("CRF_FP8", "1") == "1"
NCORES = 8
RG = [[0, 1], [2, 3], [4, 5], [6, 7]]

LAST_EXEC_NS = None
_CACHE = {}


def _install_ntff_hook():
    """Best-effort registration of the axon NTFF profile hook (image antenv
    lacks axon_hooks, so trn_boot could not register it)."""
    try:
        import types

        if "antenv.axon_hooks" in sys.modules:
            return
        holder = [None]
        m = types.ModuleType("antenv.axon_hooks")
        m.set_axon_ntff_profile_hook = lambda h: holder.__setitem__(0, h)
        m.get_axon_ntff_profile_hook = lambda: holder[0]
        sys.modules["antenv.axon_hooks"] = m
        import antenv

        antenv.axon_hooks = m
        from trn_agent_boot.trn_boot import _ntff_profile_via_ctypes

        m.set_axon_ntff_profile_hook(
            _ntff_profile_via_ctypes("/opt/axon/libaxon_pjrt.so")
        )
    except Exception:
        pass


def _build(niters, use_fp8):
    from concourse import bacc, bass, tile, mybir

    fp32, fp16 = mybir.dt.float32, mybir.dt.float16
    sdt = mybir.dt.float8e4 if use_fp8 else fp16
    qpad = QPAD if use_fp8 else D
    AF = mybir.ActivationFunctionType
    ALU = mybir.AluOpType

    nc = bacc.Bacc(None, target_bir_lowering=False)

    w_in = nc.dram_tensor("w", (NH, N), fp32, kind="ExternalInput")
    segt_in = nc.dram_tensor("segt", (128, NT, D), fp32, kind="ExternalInput")
    wt_in = nc.dram_tensor("wt", (D, D), fp32, kind="ExternalInput")
    sel_in = nc.dram_tensor("sel", (128, 2), fp32, kind="ExternalInput")
    id_in = nc.dram_tensor("ident", (128, 128), fp32, kind="ExternalInput")
    out_t = nc.dram_tensor("out", (128, NTO, D), fp32, kind="ExternalOutput")

    n_ex = max(0, niters - 1)
    cc_ins = [
        nc.dram_tensor(f"cc_in{k}", (128, NTO * qpad), sdt, kind="Internal")
        for k in range(n_ex)
    ]
    cc_outs = [
        nc.dram_tensor(f"cc_out{k}", (2, 128, NTO * qpad), sdt, kind="Internal")
        for k in range(n_ex)
    ]

    with tile.TileContext(nc) as tc:
        with (
            tc.tile_pool(name="wt_res", bufs=1) as wt_res,
            tc.tile_pool(name="slab32", bufs=2) as slab32p,
            tc.tile_pool(name="slab8", bufs=2) as slab8p,
            tc.tile_pool(name="state", bufs=1) as state,
            tc.tile_pool(name="qt", bufs=2) as qtp,
            tc.tile_pool(name="work", bufs=2) as work,
            tc.tile_pool(name="ps_mm", bufs=1, space=bass.MemorySpace.PSUM) as ps_mm,
            tc.tile_pool(name="ps_misc", bufs=3, space=bass.MemorySpace.PSUM) as ps_misc,
        ):
            # ---- small inputs (ACT-ring DMAs; slab DMAs own the SP ring) --
            id32 = state.tile([128, 128], fp32)
            nc.scalar.dma_start(id32[:], id_in[:])
            id_s = state.tile([128, 128], sdt)
            nc.vector.tensor_copy(id_s[:], id32[:])
            wt32 = state.tile([D, D], fp32)
            nc.scalar.dma_start(wt32[:], wt_in[:])
            wt16 = state.tile([D, D], fp16)
            nc.vector.tensor_copy(wt16[:], wt32[:])
            segt = state.tile([128, NT, D], fp32)
            nc.scalar.dma_start(segt[:], segt_in[:])
            selt = state.tile([128, 2], fp32)
            nc.scalar.dma_start(selt[:], sel_in[:])
            zbias = state.tile([128, 1], fp32)
            nc.vector.memset(zbias[:], 0.0)

            # ---- phase 1: load W half, cast (+rowsum) on ACT, transpose --
            if use_fp8:
                # wt_mc[mc][p, t2, i, j] = W^T[256*t2 + 128*i + p, 512*mc + j]
                wt_mc = [
                    wt_res.tile([128, NT2, 2, 512], sdt, tag=f"wtr{mc}", name=f"wt_mc{mc}")
                    for mc in range(4)
                ]
            else:
                wt_mc = [
                    wt_res.tile([128, NT, 512], sdt, tag=f"wtr{mc}", name=f"wt_mc{mc}")
                    for mc in range(4)
                ]
            rs_col = state.tile([128, SLABS], fp32)
            for ms in range(SLABS):
                w32 = slab32p.tile([128, N], fp32, tag="w32", name=f"w32_{ms}")
                nc.sync.dma_start(w32[:], w_in[ms * 128 : (ms + 1) * 128, :])
                w8 = slab8p.tile([128, N], sdt, tag="w8", name=f"w8_{ms}")
                nc.scalar.activation(
                    w8[:], w32[:], AF.Copy, accum_out=rs_col[:, ms : ms + 1]
                )
                mc, col = ms // 4, (ms % 4) * 128
                for g in range(8):
                    ptp = ps_misc.tile([128, 512], fp32, tag="misc", name=f"ptp{ms}_{g}")
                    for k2 in range(4):
                        nt = 4 * g + k2
                        nc.tensor.matmul(
                            ptp[:, k2 * 128 : (k2 + 1) * 128],
                            w8[:, nt * 128 : (nt + 1) * 128],
                            id_s[:],
                            start=True,
                            stop=True,
                        )
                    if use_fp8:
                        dst = wt_mc[mc][:, 2 * g : 2 * g + 2, :, col : col + 128]
                        src = ptp[:].rearrange("p (a b c) -> p a b c", a=2, b=2)
                    else:
                        dst = wt_mc[mc][:, 4 * g : 4 * g + 4, col : col + 128]
                        src = ptp[:].rearrange("p (a b) -> p a b", a=4)
                    nc.vector.tensor_copy(dst, src)
            # per-position reciprocal row sums, laid out [m_low=128, tile j]
            rs_rec = state.tile([128, SLABS], fp32)
            nc.vector.reciprocal(rs_rec[:], rs_col[:])

            # ---- initial Q = softmax(uniqs) over all 32 tiles ------------
            ex0 = state.tile([128, NT, D], fp32)
            nc.scalar.activation(ex0[:], segt[:], AF.Exp, bias=zbias[:])
            ssum0 = state.tile([128, NT], fp32)
            nc.vector.reduce_sum(ssum0[:], ex0[:], axis=mybir.AxisListType.X)
            srecip0 = state.tile([128, NT], fp32)
            nc.vector.reciprocal(srecip0[:], ssum0[:])
            qt_own = qtp.tile([128, NTO, qpad], sdt, tag="qt_own", name="qt_own0")
            qt_par = qtp.tile([128, NTO, qpad], sdt, tag="qt_par", name="qt_par0")
            nc.vector.tensor_tensor(
                qt_own[:, :, 0:D],
                ex0[:, 0:NTO, :],
                srecip0[:, 0:NTO, None].broadcast_to((128, NTO, D)),
                ALU.mult,
            )
            nc.vector.tensor_tensor(
                qt_par[:, :, 0:D],
                ex0[:, NTO:NT, :],
                srecip0[:, NTO:NT, None].broadcast_to((128, NTO, D)),
                ALU.mult,
            )

            # ---- phase 2: the mean-field iterations ----------------------
            for it in range(niters):
                pP = ps_mm.tile([D, NH], fp32, tag="pp", name=f"pp{it}")
                if use_fp8:
                    for t2 in range(NT2):
                        src = qt_own if t2 < NT2 // 2 else qt_par
                        j2 = t2 % (NT2 // 2)
                        lhs = src[:, 2 * j2 : 2 * j2 + 2, 0:D]
                        for mc in range(4):
                            nc.tensor.matmul(
                                pP[:, mc * 512 : (mc + 1) * 512],
                                lhs,
                                wt_mc[mc][:, t2, :, :],
                                start=(t2 == 0),
                                stop=(t2 == NT2 - 1),
                                perf_mode=mybir.MatmulPerfMode.DoubleRow,
                            )
                else:
                    for idx in range(NT):
                        src = qt_own if idx < NTO else qt_par
                        lhs = src[:, idx % NTO, :]
                        for mc in range(4):
                            nc.tensor.matmul(
                                pP[:, mc * 512 : (mc + 1) * 512],
                                lhs,
                                wt_mc[mc][:, idx, :],
                                start=(idx == 0),
                                stop=(idx == NT - 1),
                            )
                # evacuate seg_diff (unnormalized) -> fp16
                ps16 = work.tile([D, NH], fp16, tag="ps16", name=f"ps16_{it}")
                for mc in range(4):
                    sl = slice(mc * 512, (mc + 1) * 512)
                    nc.vector.tensor_copy(ps16[:, sl], pP[:, sl])
                # seg_update^T for own tiles: psum[m128, d] = Ps.T @ weights^T
                pUT = ps_misc.tile([128, NTO * D], fp32, tag="misc", name=f"pUT{it}")
                for j in range(NTO):
                    nc.tensor.matmul(
                        pUT[:, j * D : (j + 1) * D],
                        ps16[:, j * 128 : (j + 1) * 128],
                        wt16[:],
                        start=True,
                        stop=True,
                    )
                # qVals = uniqs - seg_update/rowsum  (rowsum scale is per
                # partition x tile here, broadcast over the class dim)
                upd = work.tile([128, NTO, D], fp32, tag="upd", name=f"upd{it}")
                nc.vector.tensor_tensor(
                    upd[:],
                    pUT[:].rearrange("p (a b) -> p a b", a=NTO),
                    rs_rec[:, :, None].broadcast_to((128, NTO, D)),
                    ALU.mult,
                )
                qv = work.tile([128, NTO, D], fp32, tag="qv", name=f"qv{it}")
                nc.vector.tensor_tensor(
                    qv[:], segt[:, 0:NTO, :], upd[:], ALU.subtract
                )
                if it == niters - 1:
                    nc.sync.dma_start(out_t[:], qv[:])
                    continue
                # softmax over d (free dim); values are bounded, skip max-sub
                exq = work.tile([128, NTO, D], fp32, tag="exq", name=f"exq{it}")
                nc.scalar.activation(exq[:], qv[:], AF.Exp, bias=zbias[:])
                ssum = work.tile([128, NTO], fp32, tag="ssum", name=f"ssum{it}")
                nc.vector.reduce_sum(ssum[:], exq[:], axis=mybir.AxisListType.X)
                srec = work.tile([128, NTO], fp32, tag="srec", name=f"srec{it}")
                nc.vector.reciprocal(srec[:], ssum[:])
                qt_own = qtp.tile([128, NTO, qpad], sdt, tag="qt_own", name=f"qt_own{it+1}")
                nc.vector.tensor_tensor(
                    qt_own[:, :, 0:D],
                    exq[:],
                    srec[:, :, None].broadcast_to((128, NTO, D)),
                    ALU.mult,
                )
                # pairwise exchange of the own Q half
                nc.sync.dma_start(cc_ins[it][:], qt_own[:])
                nc.gpsimd.collective_compute(
                    "AllGather",
                    ALU.bypass,
                    replica_groups=RG,
                    ins=[cc_ins[it][:].opt()],
                    outs=[cc_outs[it][:].opt()],
                )
                g0 = work.tile([128, NTO * qpad], sdt, tag="g0", name=f"g0_{it}")
                g1 = work.tile([128, NTO * qpad], sdt, tag="g1", name=f"g1_{it}")
                nc.sync.dma_start(g0[:], cc_outs[it][0][:])
                nc.sync.dma_start(g1[:], cc_outs[it][1][:])
                t0 = work.tile([128, NTO * qpad], sdt, tag="t0", name=f"t0_{it}")
                t1 = work.tile([128, NTO * qpad], sdt, tag="t1", name=f"t1_{it}")
                nc.vector.tensor_scalar_mul(t0[:], g0[:], selt[:, 0:1])
                nc.vector.tensor_scalar_mul(t1[:], g1[:], selt[:, 1:2])
                qt_par = qtp.tile([128, NTO, qpad], sdt, tag="qt_par", name=f"qt_par{it+1}")
                nc.vector.tensor_tensor(
                    qt_par[:],
                    t0[:].rearrange("p (a b) -> p a b", a=NTO),
                    t1[:].rearrange("p (a b) -> p a b", a=NTO),
                    ALU.add,
                )

    nc.compile()
    return nc


def _get_nc(niters, use_fp8):
    key = (niters, use_fp8)
    if key not in _CACHE:
        _CACHE[key] = _build(niters, use_fp8)
    return _CACHE[key]


def kernel(seg, W, weights):
    global LAST_EXEC_NS
    assert seg.shape == (BS, D, RC, RC) and W.shape == (BS, N, N)
    trace = bool(os.environ.get("BASS_TRACE"))
    if trace:
        _install_ntff_hook()

    from concourse.bass_utils import run_bass_kernel_spmd

    nc = _get_nc(NITERS, FP8)

    seg32 = np.ascontiguousarray(seg, dtype=np.float32)
    W32 = np.ascontiguousarray(W, dtype=np.float32)
    wt_np = np.ascontiguousarray(weights.T, dtype=np.float32)
    id_np = np.eye(128, dtype=np.float32)

    in_maps = []
    for c in range(NCORES):
        b, h = c // 2, c % 2
        own = slice(NH * h, NH * h + NH)
        par = slice(NH * (1 - h), NH * (1 - h) + NH)
        Wb = W32[b]
        w_np = np.ascontiguousarray(
            np.concatenate([Wb[own, own], Wb[own, par]], axis=1)
        )
        st = seg32[b].reshape(D, N).T  # [n, d]
        st_perm = np.concatenate([st[own], st[par]], axis=0)
        segt_np = np.ascontiguousarray(
            st_perm.reshape(NT, 128, D).transpose(1, 0, 2)
        )
        sel_np = np.zeros((128, 2), np.float32)
        sel_np[:, 0] = float(h)       # pick gather slot (1-h) = partner
        sel_np[:, 1] = float(1 - h)
        in_maps.append(
            {"w": w_np, "segt": segt_np, "wt": wt_np, "sel": sel_np, "ident": id_np}
        )

    res = run_bass_kernel_spmd(
        nc, in_maps, core_ids=list(range(NCORES)), trace=trace
    )
    LAST_EXEC_NS = res.exec_time_ns

    out = np.empty((BS, D, N), np.float32)
    for c in range(NCORES):
        b, h = c // 2, c % 2
        qv = res.results[c]["out"]  # [128, NTO, D]
        block = qv.transpose(2, 1, 0).reshape(D, NH)
        out[b][:, NH * h : NH * h + NH] = block
    return out.reshape(BS, D, RC, RC)


if __name__ == "__main__":
    rng = np.random.default_rng(0)
    seg = rng.standard_normal((BS, D, RC, RC)).astype(np.float32)
    W = rng.random((BS, N, N), dtype=np.float32)
    weights = rng.standard_normal((D, D)).astype(np.float32)
    out = kernel(seg=seg, W=W, weights=weights)
    print("out", out.shape, out.dtype, float(np.abs(out).mean()))


# revision 6
# speedup vs baseline: 1.2588x; 1.0333x over previous
"""CRF-RNN mean-field iteration kernel for Trainium2 (8 NeuronCores).

Math (per batch b, NITERS=5):
    D_norm = W / W.sum(axis=1, keepdims)          # row-normalized affinity [n, n]
    qVals  = uniqs = seg.reshape(d, n)
    loop:  Q = softmax(qVals, axis=0)             # over class dim d=21
           seg_diff   = Q @ D_norm^T              # [d, n]
           seg_update = weights @ seg_diff
           qVals      = uniqs - seg_update

Sharding: batch b -> core pair (2b, 2b+1); each core owns half the output
positions (m rows of W). The contraction runs over all n, so W^T (contraction
index on partitions) is built on-device via PE transpose-matmuls against an
identity, quantized to fp8-e4m3, and kept resident in SBUF across all 5
iterations -- W is read from HBM exactly once. The main matmuls run in fp8
DoubleRow mode (256-wide contraction per pass). Row-normalization (1/rowsum,
accumulated for free during the fp32->fp8 cast on the Scalar engine) is
applied per-partition to the tiny seg_update output. Per iteration the pair
exchanges its half of softmax(Q) (64 KB fp8) via a pairwise AllGather; the
instruction stream is identical on all cores (SPMD): all own/partner
asymmetry lives in host-side input permutations and a tiny select-mask input.
"""

import os
import sys

for _p in ("/opt/trn_rl_repo",):
    if _p not in sys.path:
        sys.path.insert(0, _p)

import numpy as np

BS, D, RC = 4, 21, 64
N = RC * RC       # 4096 positions
NH = N // 2       # 2048 positions per core (own half)
NT = 32           # 128-wide position tiles (global)
NTO = 16          # own tiles
NT2 = 16          # 256-wide fp8 pair tiles (global)
SLABS = 16        # own-half m slabs of 128 rows
QPAD = 32         # class-dim padding for fp8 DoubleRow lhsT stride
NITERS = int(os.environ.get("CRF_NITERS", "5"))
FP8 = os.environ.get("CRF_FP8", "1") == "1"
NCORES = 8
RG = [[0, 1], [2, 3], [4, 5], [6, 7]]

LAST_EXEC_NS = None
_CACHE = {}


def _install_ntff_hook():
    """Best-effort registration of the axon NTFF profile hook (image antenv
    lacks axon_hooks, so trn_boot could not register it)."""
    try:
        import types

        if "antenv.axon_hooks" in sys.modules:
            return
        holder = [None]
        m = types.ModuleType("antenv.axon_hooks")
        m.set_axon_ntff_profile_hook = lambda h: holder.__setitem__(0, h)
        m.get_axon_ntff_profile_hook = lambda: holder[0]
        sys.modules["antenv.axon_hooks"] = m
        import antenv

        antenv.axon_hooks = m
        from trn_agent_boot.trn_boot import _ntff_profile_via_ctypes

        m.set_axon_ntff_profile_hook(
            _ntff_profile_via_ctypes("/opt/axon/libaxon_pjrt.so")
        )
    except Exception:
        pass


def _build(niters, use_fp8):
    from concourse import bacc, bass, tile, mybir

    fp32, fp16 = mybir.dt.float32, mybir.dt.float16
    sdt = mybir.dt.float8e4 if use_fp8 else fp16
    qpad = QPAD if use_fp8 else D
    AF = mybir.ActivationFunctionType
    ALU = mybir.AluOpType

    nc = bacc.Bacc(None, target_bir_lowering=False)

    w_in = nc.dram_tensor("w", (NH, N), fp32, kind="ExternalInput")
    segt_in = nc.dram_tensor("segt", (128, NT, D), fp32, kind="ExternalInput")
    wt_in = nc.dram_tensor("wt", (D, D), fp32, kind="ExternalInput")
    sel_in = nc.dram_tensor("sel", (128, 2), fp32, kind="ExternalInput")
    id_in = nc.dram_tensor("ident", (128, 128), fp32, kind="ExternalInput")
    out_t = nc.dram_tensor("out", (128, NTO, D), fp32, kind="ExternalOutput")

    n_ex = max(0, niters - 1)
    cc_ins = [
        nc.dram_tensor(f"cc_in{k}", (128, NTO * qpad), sdt, kind="Internal")
        for k in range(n_ex)
    ]
    cc_outs = [
        nc.dram_tensor(f"cc_out{k}", (2, 128, NTO * qpad), sdt, kind="Internal")
        for k in range(n_ex)
    ]

    with tile.TileContext(nc) as tc:
        with (
            tc.tile_pool(name="wt_res", bufs=1) as wt_res,
            tc.tile_pool(name="slab32", bufs=2) as slab32p,
            tc.tile_pool(name="slab8", bufs=2) as slab8p,
            tc.tile_pool(name="state", bufs=1) as state,
            tc.tile_pool(name="qt", bufs=2) as qtp,
            tc.tile_pool(name="work", bufs=2) as work,
            tc.tile_pool(name="ps_mm", bufs=1, space=bass.MemorySpace.PSUM) as ps_mm,
            tc.tile_pool(name="ps_misc", bufs=3, space=bass.MemorySpace.PSUM) as ps_misc,
        ):
            # ---- small inputs (SWDGE ring; slab DMAs own the SP ring) ----
            id32 = state.tile([128, 128], fp32)
            nc.gpsimd.dma_start(id32[:], id_in[:])
            id_s = state.tile([128, 128], sdt)
            nc.gpsimd.tensor_copy(id_s[:], id32[:])
            wt32 = state.tile([D, D], fp32)
            nc.gpsimd.dma_start(wt32[:], wt_in[:])
            wt16 = state.tile([D, D], fp16)
            nc.gpsimd.tensor_copy(wt16[:], wt32[:])
            segt = state.tile([128, NT, D], fp32)
            nc.gpsimd.dma_start(segt[:], segt_in[:])
            selt = state.tile([128, 2], fp32)
            nc.gpsimd.dma_start(selt[:], sel_in[:])
            zbias = state.tile([128, 1], fp32)
            nc.gpsimd.memset(zbias[:], 0.0)

            # ---- phase 1: load W half, cast (+rowsum) on ACT, transpose --
            if use_fp8:
                # wt_mc[mc][p, t2, i, j] = W^T[256*t2 + 128*i + p, 512*mc + j]
                wt_mc = [
                    wt_res.tile([128, NT2, 2, 512], sdt, tag=f"wtr{mc}", name=f"wt_mc{mc}")
                    for mc in range(4)
                ]
            else:
                wt_mc = [
                    wt_res.tile([128, NT, 512], sdt, tag=f"wtr{mc}", name=f"wt_mc{mc}")
                    for mc in range(4)
                ]
            rs_col = state.tile([128, SLABS], fp32)
            for ms in range(SLABS):
                w32 = slab32p.tile([128, N], fp32, tag="w32", name=f"w32_{ms}")
                nc.sync.dma_start(w32[:], w_in[ms * 128 : (ms + 1) * 128, :])
                w8 = slab8p.tile([128, N], sdt, tag="w8", name=f"w8_{ms}")
                nc.scalar.activation(
                    w8[:], w32[:], AF.Copy, accum_out=rs_col[:, ms : ms + 1]
                )
                mc, col = ms // 4, (ms % 4) * 128
                for g in range(8):
                    ptp = ps_misc.tile([128, 512], fp32, tag="misc", name=f"ptp{ms}_{g}")
                    for k2 in range(4):
                        nt = 4 * g + k2
                        nc.tensor.matmul(
                            ptp[:, k2 * 128 : (k2 + 1) * 128],
                            w8[:, nt * 128 : (nt + 1) * 128],
                            id_s[:],
                            start=True,
                            stop=True,
                        )
                    if use_fp8:
                        dst = wt_mc[mc][:, 2 * g : 2 * g + 2, :, col : col + 128]
                        src = ptp[:].rearrange("p (a b c) -> p a b c", a=2, b=2)
                    else:
                        dst = wt_mc[mc][:, 4 * g : 4 * g + 4, col : col + 128]
                        src = ptp[:].rearrange("p (a b) -> p a b", a=4)
                    nc.vector.tensor_copy(dst, src)
            # per-position reciprocal row sums, laid out [m_low=128, tile j]
            rs_rec = state.tile([128, SLABS], fp32)
            nc.vector.reciprocal(rs_rec[:], rs_col[:])

            # ---- initial Q = softmax(uniqs) over all 32 tiles ------------
            ex0 = state.tile([128, NT, D], fp32)
            nc.scalar.activation(ex0[:], segt[:], AF.Exp, bias=zbias[:])
            ssum0 = state.tile([128, NT], fp32)
            nc.vector.reduce_sum(ssum0[:], ex0[:], axis=mybir.AxisListType.X)
            srecip0 = state.tile([128, NT], fp32)
            nc.vector.reciprocal(srecip0[:], ssum0[:])
            qt_own = qtp.tile([128, NTO, qpad], sdt, tag="qt_own", name="qt_own0")
            qt_par = qtp.tile([128, NTO, qpad], sdt, tag="qt_par", name="qt_par0")
            nc.vector.tensor_tensor(
                qt_own[:, :, 0:D],
                ex0[:, 0:NTO, :],
                srecip0[:, 0:NTO, None].broadcast_to((128, NTO, D)),
                ALU.mult,
            )
            nc.vector.tensor_tensor(
                qt_par[:, :, 0:D],
                ex0[:, NTO:NT, :],
                srecip0[:, NTO:NT, None].broadcast_to((128, NTO, D)),
                ALU.mult,
            )

            # ---- phase 2: the mean-field iterations ----------------------
            ntile = NT2 if use_fp8 else NT
            half = ntile // 2
            perf = mybir.MatmulPerfMode.DoubleRow if use_fp8 else None

            def lhs_of(t, q_own, q_par):
                if use_fp8:
                    src = q_own if t < half else q_par
                    j2 = t % half
                    return src[:, 2 * j2 : 2 * j2 + 2, 0:D]
                src = q_own if t < half else q_par
                return src[:, t % half, :]

            def rhs_of(t, mc):
                return wt_mc[mc][:, t, :, :] if use_fp8 else wt_mc[mc][:, t, :]

            for it in range(niters):
                last = it == niters - 1
                q_own, q_par = qt_own, qt_par
                pP = ps_mm.tile([D, NH], fp32, tag="pp", name=f"pp{it}")
                if it == 0:
                    # mc-outer: each wt_mc block is complete after 4 slabs, so
                    # iteration 0 overlaps the DMA-bound transpose prepass
                    phases = [[(t, mc) for t in range(ntile)] for mc in range(4)]
                else:
                    # own tiles first across all mc (runway that hides the
                    # exchange), then partner tiles mc-outer so per-mc tails
                    # pipeline behind the remaining matmuls
                    phases = [
                        [(t, mc) for t in range(half) for mc in range(4)]
                        + [(t, 0) for t in range(half, ntile)],
                        [(t, 1) for t in range(half, ntile)],
                        [(t, 2) for t in range(half, ntile)],
                        [(t, 3) for t in range(half, ntile)],
                    ]

                ps16g = []
                pUTg = []
                qvg = []
                if not last:
                    qt_next = qtp.tile(
                        [128, NTO, qpad], sdt, tag="qt_own", name=f"qt_own{it+1}"
                    )

                def emit_evac(mc):
                    t16 = work.tile([D, 512], fp16, tag=f"ps16_{mc}", name=f"ps16_{it}_{mc}")
                    nc.vector.tensor_copy(t16[:], pP[:, mc * 512 : (mc + 1) * 512])
                    ps16g.append(t16)

                def emit_ut(g):
                    pu = ps_misc.tile([128, 4 * D], fp32, tag="misc", name=f"pUT{it}_{g}")
                    for jj in range(4):
                        j = 4 * g + jj
                        nc.tensor.matmul(
                            pu[:, jj * D : (jj + 1) * D],
                            ps16g[g][:, jj * 128 : (jj + 1) * 128],
                            wt16[:],
                            start=True,
                            stop=True,
                        )
                    pUTg.append(pu)

                def emit_tail(g):
                    sl = slice(4 * g, 4 * g + 4)
                    upd = work.tile([128, 4, D], fp32, tag=f"upd{g}", name=f"upd{it}_{g}")
                    nc.vector.tensor_tensor(
                        upd[:],
                        pUTg[g][:].rearrange("p (a b) -> p a b", a=4),
                        rs_rec[:, sl, None].broadcast_to((128, 4, D)),
                        ALU.mult,
                    )
                    qv = work.tile([128, 4, D], fp32, tag=f"qv{g}", name=f"qv{it}_{g}")
                    nc.vector.tensor_tensor(qv[:], segt[:, sl, :], upd[:], ALU.subtract)
                    qvg.append(qv)
                    if last:
                        nc.sync.dma_start(out_t[:, sl, :], qv[:])
                        return
                    exq = work.tile([128, 4, D], fp32, tag=f"exq{g}", name=f"exq{it}_{g}")
                    nc.scalar.activation(exq[:], qv[:], AF.Exp, bias=zbias[:])
                    ssum = work.tile([128, 4], fp32, tag=f"ssum{g}", name=f"ssum{it}_{g}")
                    nc.vector.reduce_sum(ssum[:], exq[:], axis=mybir.AxisListType.X)
                    srec = work.tile([128, 4], fp32, tag=f"srec{g}", name=f"srec{it}_{g}")
                    nc.vector.reciprocal(srec[:], ssum[:])
                    nc.vector.tensor_tensor(
                        qt_next[:, sl, 0:D],
                        exq[:],
                        srec[:, :, None].broadcast_to((128, 4, D)),
                        ALU.mult,
                    )

                for ph in range(4):
                    for t, mc in phases[ph]:
                        nc.tensor.matmul(
                            pP[:, mc * 512 : (mc + 1) * 512],
                            lhs_of(t, q_own, q_par),
                            rhs_of(t, mc),
                            start=(t == 0),
                            stop=(t == ntile - 1),
                            perf_mode=perf,
                        )
                    emit_evac(ph)
                    if ph >= 1:
                        emit_ut(ph - 1)
                        emit_tail(ph - 1)
                emit_ut(3)
                emit_tail(3)
                if last:
                    continue

                # pairwise exchange of the own Q half
                nc.sync.dma_start(cc_ins[it][:], qt_next[:])
                nc.gpsimd.collective_compute(
                    "AllGather",
                    ALU.bypass,
                    replica_groups=RG,
                    ins=[cc_ins[it][:].opt()],
                    outs=[cc_outs[it][:].opt()],
                )
                g0 = work.tile([128, NTO * qpad], sdt, tag="g0", name=f"g0_{it}")
                g1 = work.tile([128, NTO * qpad], sdt, tag="g1", name=f"g1_{it}")
                nc.sync.dma_start(g0[:], cc_outs[it][0][:])
                nc.sync.dma_start(g1[:], cc_outs[it][1][:])
                t0 = work.tile([128, NTO * qpad], sdt, tag="t0", name=f"t0_{it}")
                t1 = work.tile([128, NTO * qpad], sdt, tag="t1", name=f"t1_{it}")
                nc.vector.tensor_scalar_mul(t0[:], g0[:], selt[:, 0:1])
                nc.vector.tensor_scalar_mul(t1[:], g1[:], selt[:, 1:2])
                qt_par_next = qtp.tile(
                    [128, NTO, qpad], sdt, tag="qt_par", name=f"qt_par{it+1}"
                )
                nc.vector.tensor_tensor(
                    qt_par_next[:],
                    t0[:].rearrange("p (a b) -> p a b", a=NTO),
                    t1[:].rearrange("p (a b) -> p a b", a=NTO),
                    ALU.add,
                )
                qt_own, qt_par = qt_next, qt_par_next

    nc.compile()
    return nc


def _get_nc(niters, use_fp8):
    key = (niters, use_fp8)
    if key not in _CACHE:
        _CACHE[key] = _build(niters, use_fp8)
    return _CACHE[key]


def kernel(seg, W, weights):
    global LAST_EXEC_NS
    assert seg.shape == (BS, D, RC, RC) and W.shape == (BS, N, N)
    trace = bool(os.environ.get("BASS_TRACE"))
    if trace:
        _install_ntff_hook()

    from concourse.bass_utils import run_bass_kernel_spmd

    nc = _get_nc(NITERS, FP8)

    seg32 = np.ascontiguousarray(seg, dtype=np.float32)
    W32 = np.ascontiguousarray(W, dtype=np.float32)
    wt_np = np.ascontiguousarray(weights.T, dtype=np.float32)
    id_np = np.eye(128, dtype=np.float32)

    in_maps = []
    for c in range(NCORES):
        b, h = c // 2, c % 2
        own = slice(NH * h, NH * h + NH)
        par = slice(NH * (1 - h), NH * (1 - h) + NH)
        Wb = W32[b]
        w_np = np.ascontiguousarray(
            np.concatenate([Wb[own, own], Wb[own, par]], axis=1)
        )
        st = seg32[b].reshape(D, N).T  # [n, d]
        st_perm = np.concatenate([st[own], st[par]], axis=0)
        segt_np = np.ascontiguousarray(
            st_perm.reshape(NT, 128, D).transpose(1, 0, 2)
        )
        sel_np = np.zeros((128, 2), np.float32)
        sel_np[:, 0] = float(h)       # pick gather slot (1-h) = partner
        sel_np[:, 1] = float(1 - h)
        in_maps.append(
            {"w": w_np, "segt": segt_np, "wt": wt_np, "sel": sel_np, "ident": id_np}
        )

    res = run_bass_kernel_spmd(
        nc, in_maps, core_ids=list(range(NCORES)), trace=trace
    )
    LAST_EXEC_NS = res.exec_time_ns

    out = np.empty((BS, D, N), np.float32)
    for c in range(NCORES):
        b, h = c // 2, c % 2
        qv = res.results[c]["out"]  # [128, NTO, D]
        block = qv.transpose(2, 1, 0).reshape(D, NH)
        out[b][:, NH * h : NH * h + NH] = block
    return out.reshape(BS, D, RC, RC)


if __name__ == "__main__":
    rng = np.random.default_rng(0)
    seg = rng.standard_normal((BS, D, RC, RC)).astype(np.float32)
    W = rng.random((BS, N, N), dtype=np.float32)
    weights = rng.standard_normal((D, D)).astype(np.float32)
    out = kernel(seg=seg, W=W, weights=weights)
    print("out", out.shape, out.dtype, float(np.abs(out).mean()))


# revision 8
# speedup vs baseline: 1.2738x; 1.0119x over previous
"""CRF-RNN mean-field iteration kernel for Trainium2 (8 NeuronCores).

Math (per batch b, NITERS=5):
    D_norm = W / W.sum(axis=1, keepdims)          # row-normalized affinity [n, n]
    qVals  = uniqs = seg.reshape(d, n)
    loop:  Q = softmax(qVals, axis=0)             # over class dim d=21
           seg_diff   = Q @ D_norm^T              # [d, n]
           seg_update = weights @ seg_diff
           qVals      = uniqs - seg_update

Sharding: batch b -> core pair (2b, 2b+1); each core owns half the output
positions (m rows of W). The contraction runs over all n, so W^T (contraction
index on partitions) is built on-device via PE transpose-matmuls against an
identity, quantized to fp8-e4m3, and kept resident in SBUF across all 5
iterations -- W is read from HBM exactly once. The main matmuls run in fp8
DoubleRow mode (256-wide contraction per pass). Row-normalization (1/rowsum,
accumulated for free during the fp32->fp8 cast on the Scalar engine) is
applied per-partition to the tiny seg_update output. Iteration 0 is emitted
interleaved with the (DMA-bound) transpose prepass so its matmuls hide under
the HBM reads. Per iteration the pair exchanges its half of softmax(Q)
(64 KB fp8) via a pairwise AllGather; the instruction stream is identical on
all cores (SPMD): all own/partner asymmetry lives in host-side input
permutations and a tiny select-mask input.
"""

import os
import sys

for _p in ("/opt/trn_rl_repo",):
    if _p not in sys.path:
        sys.path.insert(0, _p)

import numpy as np

BS, D, RC = 4, 21, 64
N = RC * RC       # 4096 positions
NH = N // 2       # 2048 positions per core (own half)
NT = 32           # 128-wide position tiles (global)
NTO = 16          # own tiles
NT2 = 16          # 256-wide fp8 pair tiles (global)
SLABS = 16        # own-half m slabs of 128 rows
QPAD = 32         # class-dim padding for fp8 DoubleRow lhsT stride
NITERS = int(os.environ.get("CRF_NITERS", "5"))
FP8 = os.environ.get("CRF_FP8", "1") == "1"
NCORES = 8
RG = [[0, 1], [2, 3], [4, 5], [6, 7]]

LAST_EXEC_NS = None
_CACHE = {}


def _install_ntff_hook():
    """Best-effort registration of the axon NTFF profile hook (image antenv
    lacks axon_hooks, so trn_boot could not register it)."""
    try:
        import types

        if "antenv.axon_hooks" in sys.modules:
            return
        holder = [None]
        m = types.ModuleType("antenv.axon_hooks")
        m.set_axon_ntff_profile_hook = lambda h: holder.__setitem__(0, h)
        m.get_axon_ntff_profile_hook = lambda: holder[0]
        sys.modules["antenv.axon_hooks"] = m
        import antenv

        antenv.axon_hooks = m
        from trn_agent_boot.trn_boot import _ntff_profile_via_ctypes

        m.set_axon_ntff_profile_hook(
            _ntff_profile_via_ctypes("/opt/axon/libaxon_pjrt.so")
        )
    except Exception:
        pass


def _build(niters, use_fp8):
    from concourse import bacc, bass, tile, mybir

    fp32, fp16 = mybir.dt.float32, mybir.dt.float16
    sdt = mybir.dt.float8e4 if use_fp8 else fp16
    qpad = QPAD if use_fp8 else D
    AF = mybir.ActivationFunctionType
    ALU = mybir.AluOpType
    ntile = NT2 if use_fp8 else NT
    half = ntile // 2
    perf = mybir.MatmulPerfMode.DoubleRow if use_fp8 else None

    nc = bacc.Bacc(None, target_bir_lowering=False)

    w_in = nc.dram_tensor("w", (NH, N), fp32, kind="ExternalInput")
    segt_in = nc.dram_tensor("segt", (128, NT, D), fp32, kind="ExternalInput")
    wt_in = nc.dram_tensor("wt", (D, D), fp32, kind="ExternalInput")
    sel_in = nc.dram_tensor("sel", (128, 2), fp32, kind="ExternalInput")
    id_in = nc.dram_tensor("ident", (128, 128), fp32, kind="ExternalInput")
    out_t = nc.dram_tensor("out", (128, NTO, D), fp32, kind="ExternalOutput")

    n_ex = max(0, niters - 1)
    cc_ins = [
        nc.dram_tensor(f"cc_in{k}", (128, NTO * qpad), sdt, kind="Internal")
        for k in range(n_ex)
    ]
    cc_outs = [
        nc.dram_tensor(f"cc_out{k}", (2, 128, NTO * qpad), sdt, kind="Internal")
        for k in range(n_ex)
    ]

    with tile.TileContext(nc) as tc:
        with (
            tc.tile_pool(name="wt_res", bufs=1) as wt_res,
            tc.tile_pool(name="slab32", bufs=2) as slab32p,
            tc.tile_pool(name="slab8", bufs=2) as slab8p,
            tc.tile_pool(name="state", bufs=1) as state,
            tc.tile_pool(name="qt", bufs=2) as qtp,
            tc.tile_pool(name="work", bufs=2) as work,
            tc.tile_pool(name="ps_mm", bufs=1, space=bass.MemorySpace.PSUM) as ps_mm,
            tc.tile_pool(name="ps_misc", bufs=3, space=bass.MemorySpace.PSUM) as ps_misc,
        ):
            # ---- small inputs (SWDGE ring; slab DMAs own the SP ring) ----
            id32 = state.tile([128, 128], fp32)
            nc.gpsimd.dma_start(id32[:], id_in[:])
            id_s = state.tile([128, 128], sdt)
            nc.gpsimd.tensor_copy(id_s[:], id32[:])
            wt32 = state.tile([D, D], fp32)
            nc.gpsimd.dma_start(wt32[:], wt_in[:])
            wt16 = state.tile([D, D], fp16)
            nc.gpsimd.tensor_copy(wt16[:], wt32[:])
            segt = state.tile([128, NT, D], fp32)
            nc.gpsimd.dma_start(segt[:], segt_in[:])
            selt = state.tile([128, 2], fp32)
            nc.gpsimd.dma_start(selt[:], sel_in[:])
            zbias = state.tile([128, 1], fp32)
            nc.gpsimd.memset(zbias[:], 0.0)
            # mask for predicated partner select: nonzero where slot1=partner
            selmask = state.tile([128, NTO * qpad], mybir.dt.uint8)
            nc.gpsimd.tensor_scalar_mul(
                selmask[:],
                selt[:, 1:2].broadcast_to((128, NTO * qpad)),
                1.0,
            )

            # ---- initial Q = softmax(uniqs) over all 32 tiles ------------
            ex0 = state.tile([128, NT, D], fp32)
            nc.scalar.activation(ex0[:], segt[:], AF.Exp, bias=zbias[:])
            ssum0 = state.tile([128, NT], fp32)
            nc.vector.reduce_sum(ssum0[:], ex0[:], axis=mybir.AxisListType.X)
            srecip0 = state.tile([128, NT], fp32)
            nc.vector.reciprocal(srecip0[:], ssum0[:])
            qt_own = qtp.tile([128, NTO, qpad], sdt, tag="qt_own", name="qt_own0")
            qt_par = qtp.tile([128, NTO, qpad], sdt, tag="qt_par", name="qt_par0")
            nc.vector.tensor_tensor(
                qt_own[:, :, 0:D],
                ex0[:, 0:NTO, :],
                srecip0[:, 0:NTO, None].broadcast_to((128, NTO, D)),
                ALU.mult,
            )
            nc.vector.tensor_tensor(
                qt_par[:, :, 0:D],
                ex0[:, NTO:NT, :],
                srecip0[:, NTO:NT, None].broadcast_to((128, NTO, D)),
                ALU.mult,
            )

            # ---- resident W^T (fp8, pair-interleaved for DoubleRow) ------
            if use_fp8:
                # wt_mc[mc][p, t2, i, j] = W^T[256*t2 + 128*i + p, 512*mc + j]
                wt_mc = [
                    wt_res.tile([128, NT2, 2, 512], sdt, tag=f"wtr{mc}", name=f"wt_mc{mc}")
                    for mc in range(4)
                ]
            else:
                wt_mc = [
                    wt_res.tile([128, NT, 512], sdt, tag=f"wtr{mc}", name=f"wt_mc{mc}")
                    for mc in range(4)
                ]
            rs_colg = [
                state.tile([128, 4], fp32, tag=f"rscol{g}", name=f"rs_col{g}")
                for g in range(4)
            ]
            rs_recg = [
                state.tile([128, 4], fp32, tag=f"rsrec{g}", name=f"rs_rec{g}")
                for g in range(4)
            ]

            def lhs_of(t, q_own, q_par):
                src = q_own if t < half else q_par
                if use_fp8:
                    j2 = t % half
                    return src[:, 2 * j2 : 2 * j2 + 2, 0:D]
                return src[:, t % half, :]

            def rhs_of(t, mc):
                return wt_mc[mc][:, t, :, :] if use_fp8 else wt_mc[mc][:, t, :]

            class IterEmitter:
                """Emits one mean-field iteration in dependency-friendly
                pieces so matmuls, evacuations, and the softmax tail
                pipeline across engines (and, for iteration 0, interleave
                with the prepass)."""

                def __init__(self, it, q_own, q_par, last):
                    self.it, self.q_own, self.q_par, self.last = it, q_own, q_par, last
                    self.pP = ps_mm.tile([D, NH], fp32, tag="pp", name=f"pp{it}")
                    self.ps16g = []
                    self.pUTg = []
                    self.qt_next = None
                    if not last:
                        self.qt_next = qtp.tile(
                            [128, NTO, qpad], sdt, tag="qt_own", name=f"qt_own{it+1}"
                        )

                def phase(self, mms):
                    for t, mc in mms:
                        nc.tensor.matmul(
                            self.pP[:, mc * 512 : (mc + 1) * 512],
                            lhs_of(t, self.q_own, self.q_par),
                            rhs_of(t, mc),
                            start=(t == 0),
                            stop=(t == ntile - 1),
                            perf_mode=perf,
                        )

                def evac(self, mc):
                    t16 = work.tile(
                        [D, 512], fp16, tag=f"ps16_{mc}", name=f"ps16_{self.it}_{mc}"
                    )
                    nc.vector.tensor_copy(t16[:], self.pP[:, mc * 512 : (mc + 1) * 512])
                    self.ps16g.append(t16)

                def ut(self, g):
                    pu = ps_misc.tile(
                        [128, 4 * D], fp32, tag="misc", name=f"pUT{self.it}_{g}"
                    )
                    for jj in range(4):
                        nc.tensor.matmul(
                            pu[:, jj * D : (jj + 1) * D],
                            self.ps16g[g][:, jj * 128 : (jj + 1) * 128],
                            wt16[:],
                            start=True,
                            stop=True,
                        )
                    self.pUTg.append(pu)

                def tail(self, g):
                    it, sl = self.it, slice(4 * g, 4 * g + 4)
                    upd = work.tile([128, 4, D], fp32, tag=f"upd{g}", name=f"upd{it}_{g}")
                    nc.vector.tensor_tensor(
                        upd[:],
                        self.pUTg[g][:].rearrange("p (a b) -> p a b", a=4),
                        rs_recg[g][:, :, None].broadcast_to((128, 4, D)),
                        ALU.mult,
                    )
                    qv = work.tile([128, 4, D], fp32, tag=f"qv{g}", name=f"qv{it}_{g}")
                    nc.vector.tensor_tensor(qv[:], segt[:, sl, :], upd[:], ALU.subtract)
                    if self.last:
                        nc.sync.dma_start(out_t[:, sl, :], qv[:])
                        return
                    exq = work.tile([128, 4, D], fp32, tag=f"exq{g}", name=f"exq{it}_{g}")
                    nc.scalar.activation(exq[:], qv[:], AF.Exp, bias=zbias[:])
                    ssum = work.tile([128, 4], fp32, tag=f"ssum{g}", name=f"ssum{it}_{g}")
                    nc.vector.reduce_sum(ssum[:], exq[:], axis=mybir.AxisListType.X)
                    srec = work.tile([128, 4], fp32, tag=f"srec{g}", name=f"srec{it}_{g}")
                    nc.vector.reciprocal(srec[:], ssum[:])
                    nc.vector.tensor_tensor(
                        self.qt_next[:, sl, 0:D],
                        exq[:],
                        srec[:, :, None].broadcast_to((128, 4, D)),
                        ALU.mult,
                    )

                def exchange(self):
                    it = self.it
                    nc.sync.dma_start(cc_ins[it][:], self.qt_next[:])
                    nc.gpsimd.collective_compute(
                        "AllGather",
                        ALU.bypass,
                        replica_groups=RG,
                        ins=[cc_ins[it][:].opt()],
                        outs=[cc_outs[it][:].opt()],
                    )
                    qt_par_next = qtp.tile(
                        [128, NTO, qpad], sdt, tag="qt_par", name=f"qt_par{it+1}"
                    )
                    g1 = work.tile([128, NTO * qpad], sdt, tag="g1", name=f"g1_{it}")
                    nc.sync.dma_start(
                        qt_par_next[:].rearrange("p a b -> p (a b)"), cc_outs[it][0][:]
                    )
                    nc.sync.dma_start(g1[:], cc_outs[it][1][:])
                    nc.vector.copy_predicated(
                        qt_par_next[:].rearrange("p a b -> p (a b)"),
                        selmask[:],
                        g1[:],
                    )
                    return qt_par_next

            # ---- prepass (slabs, transpose, rowsum) + iteration 0 --------
            em = IterEmitter(0, qt_own, qt_par, last=(niters == 1))
            for ms in range(SLABS):
                w32 = slab32p.tile([128, N], fp32, tag="w32", name=f"w32_{ms}")
                nc.sync.dma_start(w32[:], w_in[ms * 128 : (ms + 1) * 128, :])
                w8 = slab8p.tile([128, N], sdt, tag="w8", name=f"w8_{ms}")
                nc.scalar.activation(
                    w8[:], w32[:], AF.Copy,
                    accum_out=rs_colg[ms // 4][:, ms % 4 : ms % 4 + 1],
                )
                mc, col = ms // 4, (ms % 4) * 128
                for g in range(8):
                    ptp = ps_misc.tile([128, 512], fp32, tag="misc", name=f"ptp{ms}_{g}")
                    for k2 in range(4):
                        nt = 4 * g + k2
                        nc.tensor.matmul(
                            ptp[:, k2 * 128 : (k2 + 1) * 128],
                            w8[:, nt * 128 : (nt + 1) * 128],
                            id_s[:],
                            start=True,
                            stop=True,
                        )
                    if use_fp8:
                        dst = wt_mc[mc][:, 2 * g : 2 * g + 2, :, col : col + 128]
                        src = ptp[:].rearrange("p (a b c) -> p a b c", a=2, b=2)
                    else:
                        dst = wt_mc[mc][:, 4 * g : 4 * g + 4, col : col + 128]
                        src = ptp[:].rearrange("p (a b) -> p a b", a=4)
                    nc.vector.tensor_copy(dst, src)
                if ms % 4 == 3:
                    g = ms // 4
                    nc.vector.reciprocal(rs_recg[g][:], rs_colg[g][:])
                    em.phase([(t, g) for t in range(ntile)])
                    em.evac(g)
                    if g >= 1:
                        em.ut(g - 1)
                        em.tail(g - 1)
            em.ut(3)
            em.tail(3)
            if niters > 1:
                qt_par = em.exchange()
                qt_own = em.qt_next

            # ---- iterations 1..niters-1 ---------------------------------
            for it in range(1, niters):
                em = IterEmitter(it, qt_own, qt_par, last=(it == niters - 1))
                phases = [
                    [(t, mc) for t in range(half) for mc in range(4)]
                    + [(t, 0) for t in range(half, ntile)],
                    [(t, 1) for t in range(half, ntile)],
                    [(t, 2) for t in range(half, ntile)],
                    [(t, 3) for t in range(half, ntile)],
                ]
                for ph in range(4):
                    em.phase(phases[ph])
                    em.evac(ph)
                    if ph >= 1:
                        em.ut(ph - 1)
                        em.tail(ph - 1)
                em.ut(3)
                em.tail(3)
                if it < niters - 1:
                    qt_par = em.exchange()
                    qt_own = em.qt_next

    nc.compile()
    return nc


def _get_nc(niters, use_fp8):
    key = (niters, use_fp8)
    if key not in _CACHE:
        _CACHE[key] = _build(niters, use_fp8)
    return _CACHE[key]


def kernel(seg, W, weights):
    global LAST_EXEC_NS
    assert seg.shape == (BS, D, RC, RC) and W.shape == (BS, N, N)
    trace = bool(os.environ.get("BASS_TRACE"))
    if trace:
        _install_ntff_hook()

    from concourse.bass_utils import run_bass_kernel_spmd

    nc = _get_nc(NITERS, FP8)

    seg32 = np.ascontiguousarray(seg, dtype=np.float32)
    W32 = np.ascontiguousarray(W, dtype=np.float32)
    wt_np = np.ascontiguousarray(weights.T, dtype=np.float32)
    id_np = np.eye(128, dtype=np.float32)

    in_maps = []
    for c in range(NCORES):
        b, h = c // 2, c % 2
        own = slice(NH * h, NH * h + NH)
        par = slice(NH * (1 - h), NH * (1 - h) + NH)
        Wb = W32[b]
        w_np = np.ascontiguousarray(
            np.concatenate([Wb[own, own], Wb[own, par]], axis=1)
        )
        st = seg32[b].reshape(D, N).T  # [n, d]
        st_perm = np.concatenate([st[own], st[par]], axis=0)
        segt_np = np.ascontiguousarray(
            st_perm.reshape(NT, 128, D).transpose(1, 0, 2)
        )
        sel_np = np.zeros((128, 2), np.float32)
        sel_np[:, 0] = float(h)       # gather slot (1-h) = partner
        sel_np[:, 1] = float(1 - h)
        in_maps.append(
            {"w": w_np, "segt": segt_np, "wt": wt_np, "sel": sel_np, "ident": id_np}
        )

    res = run_bass_kernel_spmd(
        nc, in_maps, core_ids=list(range(NCORES)), trace=trace
    )
    LAST_EXEC_NS = res.exec_time_ns

    out = np.empty((BS, D, N), np.float32)
    for c in range(NCORES):
        b, h = c // 2, c % 2
        qv = res.results[c]["out"]  # [128, NTO, D]
        block = qv.transpose(2, 1, 0).reshape(D, NH)
        out[b][:, NH * h : NH * h + NH] = block
    return out.reshape(BS, D, RC, RC)


if __name__ == "__main__":
    rng = np.random.default_rng(0)
    seg = rng.standard_normal((BS, D, RC, RC)).astype(np.float32)
    W = rng.random((BS, N, N), dtype=np.float32)
    weights = rng.standard_normal((D, D)).astype(np.float32)
    out = kernel(seg=seg, W=W, weights=weights)
    print("out", out.shape, out.dtype, float(np.abs(out).mean()))
